# revision 1
# baseline (speedup 1.0000x reference)
"""Trainium2 Bass kernel for nn_ContrastiveDist (supervised contrastive loss).

Math
----
The reference builds (n,n) distance/weight matrices, but the loss collapses
exactly to per-class statistics.  With classes c = 0..15, per-class count
cnt[c], feature sums C[c,:], squared-norm sums SqSum[c], global sums
Ftot / SSall:

    alpha[c] = 1/(cnt[c]-1+eps)
    beta[c]  = 1/(n-cnt[c]+eps)
    loss_i   = sq_i*P[c_i] + (Q[c_i]+M) + f_i . R[c_i]
      P[c]   = alpha*cnt - beta*(n-cnt)
      Q[c]   = alpha*SqSum[c] - beta*(SSall-SqSum[c])
      R[c,:] = 2*beta*(Ftot-C[c]) - 2*alpha*C[c]
    result   = sum(relu(loss_i)*valid_i) / max(sum(valid_i), 1)

valid_i = (cnt[c_i] >= 2) is folded into the coefficients: Raug rows of
invalid classes are zeroed, so relu(loss) = 0 there, and the valid count
comes from sum(cnt[c]*vmask[c]).  Validated to ~4e-7 rel vs the f32
reference (sim).

Distribution: every core redundantly computes the full loss (inputs are
replicated).  No collectives: a cross-core AllGather costs ~9us plus a
~45us rank-skew barrier under this dispatch path, far more than the
~15us of redundant per-core compute it would save.
"""

import numpy as np
import ml_dtypes

import concourse.bacc as bacc
import concourse.tile as tile
import concourse.mybir as mybir
from concourse.bass_utils import run_bass_kernel_spmd

N, D, K, NCORES = 8192, 128, 16, 8
T = N // 128               # 64 row-tiles of 128
W = D + 3                  # faug stride: [F(128), sq, 1, pad]
EPS, MARGIN = 1e-6, 10.0
F32 = mybir.dt.float32
BF16 = mybir.dt.bfloat16
Alu = mybir.AluOpType
Act = mybir.ActivationFunctionType
AxX = mybir.AxisListType.X

# const tensor (128, CW) f32:
#   cols 0:16   iota c (one-hot compare operand, broadcast over tiles)
#   col 16      1.0  (ones(128,1) lhsT for the final partition reduce)
#   cols 17:33  1.0  (rows 0:16 = ones(16,16) lhsT for global-sum broadcast)
CW = 34

_CACHE: dict = {}


def _build():
    if "nc" in _CACHE:
        return _CACHE["nc"]

    nc = bacc.Bacc("TRN2", target_bir_lowering=False, debug=False, num_devices=NCORES)
    fain = nc.dram_tensor("fain", [128, T * W], F32, kind="ExternalInput").ap()
    fhin = nc.dram_tensor("fhin", [128, T * W], BF16, kind="ExternalInput").ap()
    flin = nc.dram_tensor("flin", [128, T * W], BF16, kind="ExternalInput").ap()
    labrep = nc.dram_tensor("labrep", [128, T * 16], F32, kind="ExternalInput").ap()
    lab16 = nc.dram_tensor("lab16", [16, N], BF16, kind="ExternalInput").ap()
    c16b = nc.dram_tensor("c16b", [16, 1], F32, kind="ExternalInput").ap()
    cst = nc.dram_tensor("cst", [128, CW], F32, kind="ExternalInput").ap()
    res = nc.dram_tensor("res", [1, 1], F32, kind="ExternalOutput").ap()

    with tile.TileContext(nc) as tc:
        with (
            tc.tile_pool(name="sb", bufs=1) as sb,
            tc.tile_pool(name="ps", bufs=1, space="PSUM") as ps,
        ):
            # ---------------- loads ----------------
            csts = sb.tile([128, CW], F32)
            nc.sync.dma_start(csts[:], cst)
            labs = sb.tile([128, T * 16], F32)
            nc.gpsimd.dma_start(labs[:], labrep)
            lab16s = sb.tile([16, N], BF16)
            nc.gpsimd.dma_start(lab16s[:], lab16)
            c16s = sb.tile([16, 1], F32)
            nc.gpsimd.dma_start(c16s[:], c16b)

            faug = sb.tile([128, T * W], F32)
            fa3 = faug.rearrange("p (t w) -> p t w", w=W)
            faugh = sb.tile([128, W * T], BF16)
            fh3 = faugh.rearrange("p (w t) -> p w t", t=T)
            faugl = sb.tile([128, W * T], BF16)
            fl3 = faugl.rearrange("p (w t) -> p w t", t=T)
            CH = T * W // 4
            for g in range(4):  # contiguous 2D chunks, alternate queues
                eng = nc.sync if g % 2 == 0 else nc.gpsimd
                eng.dma_start(faug[:, g * CH:(g + 1) * CH],
                              fain[:, g * CH:(g + 1) * CH])
            for g in range(2):
                nc.sync.dma_start(faugh[:, g * 2 * CH:(g + 1) * 2 * CH],
                                  fhin[:, g * 2 * CH:(g + 1) * 2 * CH])
                nc.gpsimd.dma_start(faugl[:, g * 2 * CH:(g + 1) * 2 * CH],
                                    flin[:, g * 2 * CH:(g + 1) * 2 * CH])

            # ---------------- one-hots ----------------
            eohaf = sb.tile([128, T * 16], F32)
            eohf3 = eohaf.rearrange("p (t c) -> p t c", c=16)
            iota3 = csts[:, 0:16].unsqueeze(1).broadcast_to((128, T, 16))
            lab3 = labs.rearrange("p (t c) -> p t c", c=16)
            nc.vector.tensor_tensor(eohf3[:, :, :], iota3, lab3, op=Alu.is_equal)
            eoha = sb.tile([128, T * 16], BF16)
            eoh3 = eoha.rearrange("p (t c) -> p t c", c=16)
            nc.vector.tensor_copy(eoha[:], eohaf[:])
            eohT = sb.tile([16, N], BF16)
            nc.vector.tensor_scalar(eohT[:], lab16s[:], c16s[:], None,
                                    op0=Alu.is_equal)

            # ---------------- sq_i then per-class stats ----------------
            ffbig = sb.tile([128, T * D], F32)
            ff3 = ffbig.rearrange("p (t d) -> p t d", d=D)
            nc.scalar.activation(ff3[:, :, :], fa3[:, :, 0:D], Act.Square)
            sqd = sb.tile([128, T], F32)
            nc.vector.tensor_reduce(sqd[:], ff3, axis=AxX, op=Alu.add)
            nc.vector.tensor_copy(fa3[:, :, D], sqd[:])
            nc.vector.tensor_copy(fh3[:, D, :], sqd[:])
            nc.vector.scalar_tensor_tensor(fl3[:, D, :], sqd[:], 0.0,
                                           fh3[:, D, :],
                                           op0=Alu.bypass, op1=Alu.subtract)

            statsP = ps.tile([16, D + 2], F32)
            for t in range(T):
                nc.tensor.matmul(statsP[:], eoh3[:, t, :], fh3[:, 0:D + 2, t],
                                 start=(t == 0), stop=False)
                nc.tensor.matmul(statsP[:], eoh3[:, t, :], fl3[:, 0:D + 2, t],
                                 start=False, stop=(t == T - 1))
            stats = sb.tile([16, D + 2], F32)
            nc.vector.tensor_copy(stats[:], statsP[:])

            # ---------------- per-class coefficients ----------------
            C = stats[:, 0:D]
            SqS = stats[:, D:D + 1]
            cnt = stats[:, D + 1:D + 2]
            gbP = ps.tile([16, D + 2], F32)
            nc.tensor.matmul(gbP[:], csts[0:16, 17:33], stats[:],
                             start=True, stop=True)
            gb = sb.tile([16, D + 2], F32)
            nc.vector.tensor_copy(gb[:], gbP[:])
            Ftot = gb[:, 0:D]
            SSall = gb[:, D:D + 1]

            alpha = sb.tile([16, 1], F32)
            nc.vector.tensor_scalar(alpha[:], cnt, EPS - 1.0, None, op0=Alu.add)
            nc.vector.reciprocal(alpha[:], alpha[:])
            beta = sb.tile([16, 1], F32)
            nc.vector.tensor_scalar(beta[:], cnt, -1.0, float(N) + EPS,
                                    op0=Alu.mult, op1=Alu.add)
            nc.vector.reciprocal(beta[:], beta[:])
            nalpha2 = sb.tile([16, 1], F32)
            nc.vector.tensor_scalar(nalpha2[:], alpha[:], -2.0, None, op0=Alu.mult)
            beta2 = sb.tile([16, 1], F32)
            nc.vector.tensor_scalar(beta2[:], beta[:], 2.0, None, op0=Alu.mult)

            raug = sb.tile([16, D + 2], F32)
            tmpd = sb.tile([16, D], F32)
            nc.vector.tensor_tensor(tmpd[:], Ftot, C, op=Alu.subtract)
            nc.vector.tensor_scalar(tmpd[:], tmpd[:], beta2[:], None, op0=Alu.mult)
            nc.vector.scalar_tensor_tensor(raug[:, 0:D], C, nalpha2[:], tmpd[:],
                                           op0=Alu.mult, op1=Alu.add)
            nmc = sb.tile([16, 1], F32)
            nc.vector.tensor_scalar(nmc[:], cnt, -1.0, float(N),
                                    op0=Alu.mult, op1=Alu.add)
            nc.vector.tensor_tensor(nmc[:], nmc[:], beta[:], op=Alu.mult)
            nc.vector.scalar_tensor_tensor(raug[:, D:D + 1], cnt, alpha[:], nmc[:],
                                           op0=Alu.mult, op1=Alu.subtract)
            ssd = sb.tile([16, 1], F32)
            nc.vector.tensor_tensor(ssd[:], SSall, SqS, op=Alu.subtract)
            nc.vector.tensor_tensor(ssd[:], ssd[:], beta[:], op=Alu.mult)
            qa = sb.tile([16, 1], F32)
            nc.vector.scalar_tensor_tensor(qa[:], SqS, alpha[:], ssd[:],
                                           op0=Alu.mult, op1=Alu.subtract)
            nc.vector.tensor_scalar(raug[:, D + 1:D + 2], qa[:], MARGIN, None,
                                    op0=Alu.add)

            # fold validity into the coefficients: zero Raug rows of classes
            # with cnt < 2, so relu(loss) vanishes for invalid rows
            vmask = sb.tile([16, 1], F32)
            nc.vector.tensor_scalar(vmask[:], cnt, 1.5, None, op0=Alu.is_ge)
            nc.vector.tensor_scalar(raug[:], raug[:], vmask[:], None, op0=Alu.mult)

            # bf16 hi/lo split of raug -> two-chain bf16 matmul ~= fp32 exact
            rhi = sb.tile([16, D + 2], BF16)
            nc.vector.tensor_copy(rhi[:], raug[:])
            rlo32 = sb.tile([16, D + 2], F32)
            nc.vector.tensor_tensor(rlo32[:], raug[:], rhi[:], op=Alu.subtract)
            rlo = sb.tile([16, D + 2], BF16)
            nc.vector.tensor_copy(rlo[:], rlo32[:])

            # ---------------- per-row losses ----------------
            lossrows = sb.tile([128, T], F32)
            for r in range(T // 2):  # 32 rounds x 2 tiles; D-psum 2 banks x2 slots
                dP = ps.tile([128, 2 * 512], F32, tag="dpsum", bufs=2,
                             name=f"dP{r}")
                d3 = dP.rearrange("p (b x) -> p b x", x=512)
                for j in range(2):
                    t = r * 2 + j
                    lhs = eohT[:, t * 128:(t + 1) * 128]
                    nc.tensor.matmul(d3[:, j, 0:D + 2], lhs, rhi[:],
                                     start=True, stop=False)
                    nc.tensor.matmul(d3[:, j, 0:D + 2], lhs, rlo[:],
                                     start=False, stop=True)
                for j in range(2):
                    t = r * 2 + j
                    pscr = sb.tile([128, D + 2], F32, tag="pscr", bufs=4,
                                   name=f"ps{r}_{j}")
                    nc.vector.scalar_tensor_tensor(
                        pscr[:], d3[:, j, 0:D + 2], 0.0, fa3[:, t, 0:D + 2],
                        op0=Alu.bypass, op1=Alu.mult,
                        accum_out=lossrows[:, t:t + 1])

            # ---------------- final reduction ----------------
            accpair = sb.tile([128, 2], F32)
            nc.gpsimd.memset(accpair[:, 1:2], 0.0)
            relscr = sb.tile([128, T], F32)
            nc.vector.tensor_scalar(relscr[:], lossrows[:], 0.0, None,
                                    op0=Alu.max, op1=Alu.add,
                                    accum_out=accpair[:, 0:1])
            nc.vector.tensor_tensor(accpair[0:16, 1:2], cnt, vmask[:],
                                    op=Alu.mult)
            finP = ps.tile([1, 2], F32)
            nc.tensor.matmul(finP[:], csts[:, 16:17], accpair[:],
                             start=True, stop=True)
            fin = sb.tile([1, 2], F32)
            nc.vector.tensor_copy(fin[:], finP[:])
            den = sb.tile([1, 1], F32)
            nc.vector.tensor_scalar(den[:], fin[:, 1:2], 1.0, None, op0=Alu.max)
            nc.vector.reciprocal(den[:], den[:])
            resS = sb.tile([1, 1], F32)
            nc.vector.tensor_tensor(resS[:], fin[:, 0:1], den[:], op=Alu.mult)
            nc.sync.dma_start(res, resS[:])

    nc.compile()
    _CACHE["nc"] = nc
    return nc


def _make_in_maps(features, labels):
    feats = np.ascontiguousarray(np.asarray(features, dtype=np.float32))
    lab = np.ascontiguousarray(np.asarray(labels)).astype(np.float32)

    cst = np.zeros((128, CW), np.float32)
    cst[:, 0:16] = np.arange(16, dtype=np.float32)[None, :]
    cst[:, 16:33] = 1.0

    fa = np.zeros((128, T, W), np.float32)
    fa[:, :, 0:D] = feats.reshape(T, 128, D).transpose(1, 0, 2)
    fa[:, :, D + 1] = 1.0
    fawt = np.ascontiguousarray(fa.transpose(0, 2, 1))  # (128, W, T)
    fh = fawt.reshape(128, W * T).astype(ml_dtypes.bfloat16)
    fl = (fawt.reshape(128, W * T) - fh.astype(np.float32)).astype(
        ml_dtypes.bfloat16)
    fa = fa.reshape(128, T * W)

    one = {
        "fain": fa,
        "fhin": fh,
        "flin": fl,
        "labrep": np.ascontiguousarray(
            np.repeat(lab.reshape(T, 128).T, 16, axis=1)),
        "lab16": np.ascontiguousarray(
            np.broadcast_to(lab, (16, N))).astype(ml_dtypes.bfloat16),
        "c16b": np.arange(16, dtype=np.float32).reshape(16, 1),
        "cst": cst,
    }
    return [dict(one) for _ in range(NCORES)]


def kernel(features, labels):
    nc = _build()
    in_maps = _make_in_maps(features, labels)
    out = run_bass_kernel_spmd(nc, in_maps, core_ids=list(range(NCORES)))
    return np.float32(out.results[0]["res"][0, 0])



# revision 5
# speedup vs baseline: 1.5559x; 1.5559x over previous
"""Trainium2 Bass kernel for nn_ContrastiveDist (supervised contrastive loss).

Math
----
The (n,n) distance/weight matrices collapse to per-class statistics.  With
classes c = 0..15, per-class count cnt[c], feature sums C[c,:], squared-norm
sums SqS[c], global sums Ftot / SSall:

    alpha[c] = 1/(cnt[c]-1+eps),  beta[c] = 1/(n-cnt[c]+eps)
    loss_i   = f_i . R[c_i] + sq_i*P[c_i] + (Q[c_i]+M)
      R[c,:] = 2*beta*(Ftot-C[c]) - 2*alpha*C[c]
      P[c]   = alpha*cnt - beta*(n-cnt)
      Q[c]   = alpha*SqS[c] - beta*(SSall-SqS[c])
    result   = sum(relu(loss_i)*valid_i) / max(sum(valid_i), 1)

valid_i = (cnt[c_i] >= 2) is folded into the coefficients (R/P/QM rows of
invalid classes zeroed -> relu(loss)=0 there).

Device pipeline (single-chain bf16, ~4e-5 rel err vs f32 reference):
  1. stats:  statsT(128d,16c) = sum_t fh_tile^T @ onehot_tile  (64-matmul
     PSUM accumulation chain; lhsT = feature tiles so the output lands
     directly in the transposed layout needed as dot-phase weights).
  2. cnt/SqS on vector from rows-layout onehot * sq, partition-reduced by a
     ones(128,1) matmul; coefficients computed in a (1,16) free-layout frame
     and broadcast to 128 partitions with a ones(1,128) rank-1 matmul.
  3. loss:   per 512-col chunk, PSUM = RT^T @ fT + P128^T @ fT^2  (the second
     matmul realizes P[c]*sq_i since sum_d f[i,d]^2 = sq_i), then one fused
     vector op (PSUM + QM[c])*onehotT and one relu+accумulate op.
Total HBM traffic ~4.7MB/core (bf16 features in rows + transposed layouts,
prebuilt one-hots); every core computes redundantly (no collectives).
"""

import numpy as np
import ml_dtypes

import concourse.bacc as bacc
import concourse.tile as tile
import concourse.mybir as mybir
from concourse.bass_utils import run_bass_kernel_spmd

N, D, K, NCORES = 8192, 128, 16, 8
T = N // 128               # 64 row-tiles of 128
NCH = 16                   # dot chunks of 512 cols
CH = N // NCH
FCH = 8                    # DMA / square chunking
EPS, MARGIN = 1e-6, 10.0
F32 = mybir.dt.float32
BF16 = mybir.dt.bfloat16
Alu = mybir.AluOpType
Act = mybir.ActivationFunctionType
AxX = mybir.AxisListType.X

_CACHE: dict = {}


def _build():
    if "nc" in _CACHE:
        return _CACHE["nc"]

    nc = bacc.Bacc("TRN2", target_bir_lowering=False, debug=False, num_devices=NCORES)
    fhr = nc.dram_tensor("fhr", [128, T * D], BF16, kind="ExternalInput").ap()
    ftr = nc.dram_tensor("ftr", [128, N], BF16, kind="ExternalInput").ap()
    eohr = nc.dram_tensor("eohr", [128, T * K], BF16, kind="ExternalInput").ap()
    eoht = nc.dram_tensor("eoht", [K, N], BF16, kind="ExternalInput").ap()
    res = nc.dram_tensor("res", [1, 1], F32, kind="ExternalOutput").ap()

    with tile.TileContext(nc) as tc:
        with (
            tc.tile_pool(name="sb", bufs=1) as sb,
            tc.tile_pool(name="ps", bufs=1, space="PSUM") as ps,
        ):
            # ---------------- loads ----------------
            eohs = sb.tile([128, T * K], BF16)
            nc.sync.dma_start(eohs[:], eohr)
            fh = sb.tile([128, T * D], BF16)
            FC = T * D // FCH
            for g in range(FCH):
                eng = nc.sync if g % 2 == 0 else nc.gpsimd
                eng.dma_start(fh[:, g * FC:(g + 1) * FC], fhr[:, g * FC:(g + 1) * FC])
            eohts = sb.tile([K, N], BF16)
            nc.gpsimd.dma_start(eohts[:], eoht)
            ft = sb.tile([128, N], BF16)
            FT = N // FCH
            for g in range(FCH):
                eng = nc.gpsimd if g % 2 == 0 else nc.sync
                eng.dma_start(ft[:, g * FT:(g + 1) * FT], ftr[:, g * FT:(g + 1) * FT])

            fh3 = fh.rearrange("p (t d) -> p t d", d=D)
            eoh3 = eohs.rearrange("p (t c) -> p t c", c=K)
            eoh3c = eohs.rearrange("p (t c) -> p c t", c=K)

            # ---------------- sq_i = sum_d h^2 (rows layout) ----------------
            sqdb = sb.tile([128, T], BF16)
            TPC = T // FCH
            for g in range(FCH):
                fsq = sb.tile([128, TPC * D], BF16, tag="fsq", bufs=2, name=f"fsq{g}")
                fsq3 = fsq.rearrange("p (t d) -> p t d", d=D)
                nc.scalar.activation(fsq[:], fh[:, g * FC:(g + 1) * FC], Act.Square)
                with nc.allow_low_precision(reason="bf16 sq feeds SqS only"):
                    nc.vector.tensor_reduce(sqdb[:, g * TPC:(g + 1) * TPC], fsq3,
                                            axis=AxX, op=Alu.add)

            # ---------------- ft2 = fT^2 (transposed layout) ----------------
            ft2 = sb.tile([128, N], BF16)
            for g in range(FCH):
                nc.scalar.activation(ft2[:, g * FT:(g + 1) * FT],
                                     ft[:, g * FT:(g + 1) * FT], Act.Square)

            # ---------------- statsT accumulation chain ----------------
            statsP = ps.tile([128, K], F32)
            for t in range(T):
                nc.tensor.matmul(statsP[:], fh3[:, t, :], eoh3[:, t, :],
                                 start=(t == 0), stop=(t == T - 1))

            # ---------------- cnt / SqS (vector + ones matmul) ----------------
            prods = sb.tile([128, K * T], F32)
            prods3 = prods.rearrange("p (c t) -> p c t", t=T)
            sqb3 = sqdb.unsqueeze(1).broadcast_to((128, K, T))
            nc.vector.tensor_tensor(prods3[:, :, :], eoh3c, sqb3, op=Alu.mult)
            packT = sb.tile([128, 2 * K], F32)
            nc.vector.tensor_reduce(packT[:, 0:K], prods3, axis=AxX, op=Alu.add)
            nc.vector.tensor_reduce(packT[:, K:2 * K], eoh3c, axis=AxX, op=Alu.add)
            ones128 = sb.tile([128, 1], F32)
            nc.gpsimd.memset(ones128[:], 1.0)
            csP = ps.tile([1, 2 * K], F32)
            nc.tensor.matmul(csP[:], ones128[:], packT[:], start=True, stop=True)
            cs = sb.tile([1, 2 * K], F32)
            nc.vector.tensor_copy(cs[:], csP[:])
            SqS = cs[:, 0:K]
            cntf = cs[:, K:2 * K]

            # ---------------- per-class coefficients (1, K) frame ----------------
            alpha = sb.tile([1, K], F32)
            nc.vector.tensor_scalar(alpha[:], cntf, EPS - 1.0, None, op0=Alu.add)
            nc.vector.reciprocal(alpha[:], alpha[:])
            beta = sb.tile([1, K], F32)
            nc.vector.tensor_scalar(beta[:], cntf, -1.0, float(N) + EPS,
                                    op0=Alu.mult, op1=Alu.add)
            nc.vector.reciprocal(beta[:], beta[:])
            vmask = sb.tile([1, K], F32)
            nc.vector.tensor_scalar(vmask[:], cntf, 1.5, None, op0=Alu.is_ge)

            ssall = sb.tile([1, 1], F32)
            nc.vector.tensor_reduce(ssall[:], SqS, axis=AxX, op=Alu.add)
            t1 = sb.tile([1, K], F32)
            nc.vector.scalar_tensor_tensor(t1[:], SqS, -1.0,
                                           ssall.broadcast_to((1, K)),
                                           op0=Alu.mult, op1=Alu.add)  # SSall-SqS
            nc.vector.tensor_tensor(t1[:], t1[:], beta[:], op=Alu.mult)
            qm = sb.tile([1, K], F32)
            nc.vector.tensor_tensor(qm[:], SqS, alpha[:], op=Alu.mult)
            nc.vector.tensor_tensor(qm[:], qm[:], t1[:], op=Alu.subtract)
            nc.vector.tensor_scalar(qm[:], qm[:], MARGIN, None, op0=Alu.add)
            nc.vector.tensor_tensor(qm[:], qm[:], vmask[:], op=Alu.mult)

            nmc = sb.tile([1, K], F32)
            nc.vector.tensor_scalar(nmc[:], cntf, -1.0, float(N),
                                    op0=Alu.mult, op1=Alu.add)        # N-cnt
            nc.vector.tensor_tensor(nmc[:], nmc[:], beta[:], op=Alu.mult)
            pf = sb.tile([1, K], F32)
            nc.vector.tensor_tensor(pf[:], cntf, alpha[:], op=Alu.mult)
            nc.vector.tensor_tensor(pf[:], pf[:], nmc[:], op=Alu.subtract)
            nc.vector.tensor_tensor(pf[:], pf[:], vmask[:], op=Alu.mult)

            cpack = sb.tile([1, 3 * K], F32)
            nc.vector.tensor_scalar(cpack[:, 0:K], beta[:], 2.0, None, op0=Alu.mult)
            nc.vector.tensor_tensor(cpack[:, 0:K], cpack[:, 0:K], vmask[:],
                                    op=Alu.mult)                       # 2b*vm
            nc.vector.tensor_scalar(cpack[:, K:2 * K], alpha[:], -2.0, None,
                                    op0=Alu.mult)
            nc.vector.tensor_tensor(cpack[:, K:2 * K], cpack[:, K:2 * K], vmask[:],
                                    op=Alu.mult)                       # -2a*vm
            nc.vector.tensor_copy(cpack[:, 2 * K:3 * K], pf[:])        # P

            ones1 = sb.tile([1, 128], F32)
            nc.gpsimd.memset(ones1[:], 1.0)
            bcP = ps.tile([128, 3 * K], F32)
            nc.tensor.matmul(bcP[:], ones1[:], cpack[:], start=True, stop=True)

            qmtP = ps.tile([K, 1], F32)
            nc.tensor.matmul(qmtP[:], qm[:], ones1[:, 0:1], start=True, stop=True)
            qm16 = sb.tile([K, 1], F32)
            nc.vector.tensor_copy(qm16[:], qmtP[:])

            statsS = sb.tile([128, K], F32)
            nc.vector.tensor_copy(statsS[:], statsP[:])
            ftot = sb.tile([128, 1], F32)
            nc.vector.tensor_reduce(ftot[:], statsS[:], axis=AxX, op=Alu.add)
            rtf = sb.tile([128, K], F32)
            nc.vector.tensor_tensor(rtf[:], ftot.broadcast_to((128, K)), statsS[:],
                                    op=Alu.subtract)                   # Ftot-C^T
            nc.vector.tensor_tensor(rtf[:], rtf[:], bcP[:, 0:K], op=Alu.mult)
            tmp2 = sb.tile([128, K], F32)
            nc.vector.tensor_tensor(tmp2[:], statsS[:], bcP[:, K:2 * K], op=Alu.mult)
            nc.vector.tensor_tensor(rtf[:], rtf[:], tmp2[:], op=Alu.add)
            rts = sb.tile([128, K], BF16)
            nc.vector.tensor_copy(rts[:], rtf[:])
            p128s = sb.tile([128, K], BF16)
            nc.vector.tensor_copy(p128s[:], bcP[:, 2 * K:3 * K])

            # ---------------- loss chunks ----------------
            partials = sb.tile([K, NCH], F32)
            for k in range(NCH):
                dP = ps.tile([K, CH], F32, tag="dpsum", bufs=2, name=f"dP{k}")
                nc.tensor.matmul(dP[:], rts[:], ft[:, k * CH:(k + 1) * CH],
                                 start=True, stop=False)
                nc.tensor.matmul(dP[:], p128s[:], ft2[:, k * CH:(k + 1) * CH],
                                 start=False, stop=True)
                mskd = sb.tile([K, CH], F32, tag="mskd", bufs=2, name=f"m{k}")
                nc.vector.scalar_tensor_tensor(mskd[:], dP[:], qm16[:],
                                               eohts[:, k * CH:(k + 1) * CH],
                                               op0=Alu.add, op1=Alu.mult)
                scr = sb.tile([K, CH], BF16, tag="scr", bufs=2, name=f"s{k}")
                nc.vector.tensor_scalar(scr[:], mskd[:], 0.0, None, op0=Alu.max,
                                        op1=Alu.add,
                                        accum_out=partials[:, k:k + 1])

            # ---------------- final reduction ----------------
            numP = ps.tile([1, NCH], F32)
            nc.tensor.matmul(numP[:], ones128[0:K, :], partials[:],
                             start=True, stop=True)
            num = sb.tile([1, 1], F32)
            nc.vector.tensor_reduce(num[:], numP[:], axis=AxX, op=Alu.add)
            dv = sb.tile([1, K], F32)
            nc.vector.tensor_tensor(dv[:], cntf, vmask[:], op=Alu.mult)
            den = sb.tile([1, 1], F32)
            nc.vector.tensor_reduce(den[:], dv[:], axis=AxX, op=Alu.add)
            nc.vector.tensor_scalar(den[:], den[:], 1.0, None, op0=Alu.max)
            nc.vector.reciprocal(den[:], den[:])
            resS = sb.tile([1, 1], F32)
            nc.vector.tensor_tensor(resS[:], num[:], den[:], op=Alu.mult)
            nc.sync.dma_start(res, resS[:])

    nc.compile()
    _CACHE["nc"] = nc
    return nc


def _make_in_maps(features, labels):
    feats = np.ascontiguousarray(np.asarray(features, dtype=np.float32))
    lab = np.ascontiguousarray(np.asarray(labels)).astype(np.int64)
    bf = ml_dtypes.bfloat16

    oh = (lab[:, None] == np.arange(K, dtype=np.int64)[None, :]).astype(bf)  # (N, K)
    one = {
        "fhr": np.ascontiguousarray(
            feats.reshape(T, 128, D).transpose(1, 0, 2).reshape(128, T * D)
        ).astype(bf),
        "ftr": np.ascontiguousarray(feats.T).astype(bf),
        "eohr": np.ascontiguousarray(
            oh.reshape(T, 128, K).transpose(1, 0, 2).reshape(128, T * K)),
        "eoht": np.ascontiguousarray(oh.T),
    }
    return [dict(one) for _ in range(NCORES)]


def kernel(features, labels):
    nc = _build()
    in_maps = _make_in_maps(features, labels)
    out = run_bass_kernel_spmd(nc, in_maps, core_ids=list(range(NCORES)))
    return np.float32(out.results[0]["res"][0, 0])


# revision 8
# speedup vs baseline: 1.7443x; 1.1210x over previous
"""Trainium2 Bass kernel for nn_ContrastiveDist (supervised contrastive loss).

Math
----
The (n,n) distance/weight matrices collapse to per-class statistics.  With
classes c = 0..15, per-class count cnt[c], feature sums C[c,:], squared-norm
sums SqS[c], global sums Ftot / SSall:

    alpha[c] = 1/(cnt[c]-1+eps),  beta[c] = 1/(n-cnt[c]+eps)
    loss_i   = f_i . R[c_i] + sq_i*P[c_i] + (Q[c_i]+M)
      R[c,:] = 2*beta*(Ftot-C[c]) - 2*alpha*C[c]
      P[c]   = alpha*cnt - beta*(n-cnt)
      Q[c]   = alpha*SqS[c] - beta*(SSall-SqS[c])
    result   = sum(relu(loss_i)*valid_i) / max(sum(valid_i), 1)

valid_i = (cnt[c_i] >= 2) is folded into the coefficients (R/P/QM rows of
invalid classes zeroed -> relu(loss)=0 there).

Device pipeline (single-chain bf16, ~5e-5 rel err vs f32 reference):
  1. stats:  statsT(128d,16c) = sum_t fh_tile^T @ onehot_tile  (64-matmul
     PSUM accumulation chain, lands directly in the transposed layout needed
     as dot-phase weights), overlapped with the feature DMA.
  2. cnt/SqS on vector from rows-layout onehot * sq, partition-reduced by a
     ones(128,1) matmul; coefficients computed in a (1,16) free-layout frame
     and broadcast to 128 partitions with a ones(1,128) rank-1 matmul.
  3. loss:   per 512-col chunk, PSUM = RT^T @ fT + P128^T @ fT^2  (the second
     matmul realizes P[c]*sq_i since sum_d fT^2[d,i] = sq_i), then
     (PSUM + QM[c])*onehotT on gpsimd and relu+accumulate on vector.
Total HBM traffic ~4.7MB/core (bf16 features in rows + transposed layouts,
prebuilt one-hots); every core computes redundantly (no collectives).
DMAs ride all three dispatch rings (sync/scalar HWDGE + gpsimd SWDGE);
squares of fT run on the Pool engine so they hide under the DMA window.
"""

import numpy as np
import ml_dtypes

import concourse.bacc as bacc
import concourse.tile as tile
import concourse.mybir as mybir
from concourse.bass_utils import run_bass_kernel_spmd

N, D, K, NCORES = 8192, 128, 16, 8
T = N // 128               # 64 row-tiles of 128
NCH = 16                   # dot chunks of 512 cols
CH = N // NCH
FCH = 4                    # DMA / square chunking (2048 cols each)
EPS, MARGIN = 1e-6, 10.0
F32 = mybir.dt.float32
BF16 = mybir.dt.bfloat16
Alu = mybir.AluOpType
Act = mybir.ActivationFunctionType
AxX = mybir.AxisListType.X

_CACHE: dict = {}


def _build():
    if "nc" in _CACHE:
        return _CACHE["nc"]

    nc = bacc.Bacc("TRN2", target_bir_lowering=False, debug=False, num_devices=NCORES)
    fhr = nc.dram_tensor("fhr", [128, T * D], BF16, kind="ExternalInput").ap()
    ftr = nc.dram_tensor("ftr", [128, N], BF16, kind="ExternalInput").ap()
    eohr = nc.dram_tensor("eohr", [128, T * K], BF16, kind="ExternalInput").ap()
    eoht = nc.dram_tensor("eoht", [K, N], BF16, kind="ExternalInput").ap()
    res = nc.dram_tensor("res", [1, 1], F32, kind="ExternalOutput").ap()

    with tile.TileContext(nc) as tc:
        with (
            tc.tile_pool(name="sb", bufs=1) as sb,
            tc.tile_pool(name="ps", bufs=1, space="PSUM") as ps,
        ):
            # ---------------- loads (3 dispatch rings, fh first) ----------------
            eohs = sb.tile([128, T * K], BF16)
            fh = sb.tile([128, T * D], BF16)
            ft = sb.tile([128, N], BF16)
            eohts = sb.tile([K, N], BF16)
            FC = T * D // FCH
            FT = N // FCH
            nc.scalar.dma_start(eohs[:], eohr)
            nc.sync.dma_start(fh[:, 0 * FC:1 * FC], fhr[:, 0 * FC:1 * FC])
            nc.scalar.dma_start(fh[:, 1 * FC:2 * FC], fhr[:, 1 * FC:2 * FC])
            nc.sync.dma_start(fh[:, 2 * FC:3 * FC], fhr[:, 2 * FC:3 * FC])
            nc.scalar.dma_start(fh[:, 3 * FC:4 * FC], fhr[:, 3 * FC:4 * FC])
            nc.gpsimd.dma_start(eohts[:], eoht)
            nc.gpsimd.dma_start(ft[:, 0 * FT:1 * FT], ftr[:, 0 * FT:1 * FT])
            nc.sync.dma_start(ft[:, 1 * FT:2 * FT], ftr[:, 1 * FT:2 * FT])
            nc.scalar.dma_start(ft[:, 2 * FT:3 * FT], ftr[:, 2 * FT:3 * FT])
            nc.sync.dma_start(ft[:, 3 * FT:4 * FT], ftr[:, 3 * FT:4 * FT])

            fh3 = fh.rearrange("p (t d) -> p t d", d=D)
            eoh3 = eohs.rearrange("p (t c) -> p t c", c=K)
            eoh3c = eohs.rearrange("p (t c) -> p c t", c=K)

            # ---------------- sq_i = sum_d h^2 (rows layout) ----------------
            sqdb = sb.tile([128, T], BF16)
            TPC = T // FCH
            for g in range(FCH):
                fsq = sb.tile([128, TPC * D], BF16, tag="fsq", bufs=2, name=f"fsq{g}")
                fsq3 = fsq.rearrange("p (t d) -> p t d", d=D)
                nc.scalar.activation(fsq[:], fh[:, g * FC:(g + 1) * FC], Act.Square)
                with nc.allow_low_precision(reason="bf16 sq feeds SqS only"):
                    nc.vector.tensor_reduce(sqdb[:, g * TPC:(g + 1) * TPC], fsq3,
                                            axis=AxX, op=Alu.add)

            # ---------------- ft2 = fT^2 on Pool engine ----------------
            ft2 = sb.tile([128, N], BF16)
            for g in range(FCH):
                nc.gpsimd.tensor_tensor(ft2[:, g * FT:(g + 1) * FT],
                                        ft[:, g * FT:(g + 1) * FT],
                                        ft[:, g * FT:(g + 1) * FT], op=Alu.mult)

            # ---------------- statsT accumulation chain ----------------
            statsP = ps.tile([128, K], F32)
            for t in range(T):
                nc.tensor.matmul(statsP[:], fh3[:, t, :], eoh3[:, t, :],
                                 start=(t == 0), stop=(t == T - 1))

            # ---------------- cnt / SqS (vector + ones matmul) ----------------
            packT = sb.tile([128, 2 * K], F32)
            nc.vector.tensor_reduce(packT[:, K:2 * K], eoh3c, axis=AxX, op=Alu.add)
            prods = sb.tile([128, K * T], F32)
            prods3 = prods.rearrange("p (c t) -> p c t", t=T)
            sqb3 = sqdb.unsqueeze(1).broadcast_to((128, K, T))
            nc.vector.tensor_tensor(prods3[:, :, :], eoh3c, sqb3, op=Alu.mult)
            nc.vector.tensor_reduce(packT[:, 0:K], prods3, axis=AxX, op=Alu.add)
            ones128 = sb.tile([128, 1], F32)
            nc.gpsimd.memset(ones128[:], 1.0)
            csP = ps.tile([1, 2 * K], F32)
            nc.tensor.matmul(csP[:], ones128[:], packT[:], start=True, stop=True)
            cs = sb.tile([1, 2 * K], F32)
            nc.vector.tensor_copy(cs[:], csP[:])
            SqS = cs[:, 0:K]
            cntf = cs[:, K:2 * K]

            # ---------------- per-class coefficients (1, K) frame ----------------
            alpha = sb.tile([1, K], F32)
            nc.vector.tensor_scalar(alpha[:], cntf, EPS - 1.0, None, op0=Alu.add)
            nc.vector.reciprocal(alpha[:], alpha[:])
            beta = sb.tile([1, K], F32)
            nc.vector.tensor_scalar(beta[:], cntf, -1.0, float(N) + EPS,
                                    op0=Alu.mult, op1=Alu.add)
            nc.vector.reciprocal(beta[:], beta[:])
            vmask = sb.tile([1, K], F32)
            nc.vector.tensor_scalar(vmask[:], cntf, 1.5, None, op0=Alu.is_ge)

            ssall = sb.tile([1, 1], F32)
            nc.vector.tensor_reduce(ssall[:], SqS, axis=AxX, op=Alu.add)
            t1 = sb.tile([1, K], F32)
            nc.vector.scalar_tensor_tensor(t1[:], SqS, -1.0,
                                           ssall.broadcast_to((1, K)),
                                           op0=Alu.mult, op1=Alu.add)  # SSall-SqS
            nc.vector.tensor_tensor(t1[:], t1[:], beta[:], op=Alu.mult)
            qm = sb.tile([1, K], F32)
            nc.vector.tensor_tensor(qm[:], SqS, alpha[:], op=Alu.mult)
            nc.vector.scalar_tensor_tensor(qm[:], qm[:], MARGIN, t1[:],
                                           op0=Alu.add, op1=Alu.subtract)
            nc.vector.tensor_tensor(qm[:], qm[:], vmask[:], op=Alu.mult)

            nmc = sb.tile([1, K], F32)
            nc.vector.tensor_scalar(nmc[:], cntf, -1.0, float(N),
                                    op0=Alu.mult, op1=Alu.add)        # N-cnt
            nc.vector.tensor_tensor(nmc[:], nmc[:], beta[:], op=Alu.mult)
            pf = sb.tile([1, K], F32)
            nc.vector.tensor_tensor(pf[:], cntf, alpha[:], op=Alu.mult)
            nc.vector.tensor_tensor(pf[:], pf[:], nmc[:], op=Alu.subtract)

            cpack = sb.tile([1, 3 * K], F32)
            nc.vector.tensor_scalar(cpack[:, 0:K], beta[:], 2.0, None, op0=Alu.mult)
            nc.vector.tensor_scalar(cpack[:, K:2 * K], alpha[:], -2.0, None,
                                    op0=Alu.mult)
            nc.vector.tensor_tensor(cpack[:, 2 * K:3 * K], pf[:], vmask[:],
                                    op=Alu.mult)                       # P*vm
            # fold vmask into the R pieces via one strided multiply over [2b|-2a]
            vm2 = cpack[:, 0:2 * K].rearrange("o (a c) -> o a c", c=K)
            vmb = vmask.unsqueeze(1).broadcast_to((1, 2, K))
            nc.vector.tensor_tensor(vm2[:, :, :], vm2, vmb, op=Alu.mult)

            ones1 = sb.tile([1, 128], F32)
            nc.gpsimd.memset(ones1[:], 1.0)
            bcP = ps.tile([128, 3 * K], F32)
            nc.tensor.matmul(bcP[:], ones1[:], cpack[:], start=True, stop=True)

            qmtP = ps.tile([K, 1], F32)
            nc.tensor.matmul(qmtP[:], qm[:], ones1[:, 0:1], start=True, stop=True)
            qm16 = sb.tile([K, 1], F32)
            nc.vector.tensor_copy(qm16[:], qmtP[:])

            statsS = sb.tile([128, K], F32)
            nc.vector.tensor_copy(statsS[:], statsP[:])
            ftot = sb.tile([128, 1], F32)
            nc.vector.tensor_reduce(ftot[:], statsS[:], axis=AxX, op=Alu.add)
            rtf = sb.tile([128, K], F32)
            nc.vector.tensor_tensor(rtf[:], ftot.broadcast_to((128, K)), statsS[:],
                                    op=Alu.subtract)                   # Ftot-C^T
            nc.vector.tensor_tensor(rtf[:], rtf[:], bcP[:, 0:K], op=Alu.mult)
            tmp2 = sb.tile([128, K], F32)
            nc.vector.tensor_tensor(tmp2[:], statsS[:], bcP[:, K:2 * K], op=Alu.mult)
            rts = sb.tile([128, K], BF16)
            nc.vector.tensor_tensor(rts[:], rtf[:], tmp2[:], op=Alu.add)
            p128s = sb.tile([128, K], BF16)
            nc.vector.tensor_copy(p128s[:], bcP[:, 2 * K:3 * K])

            # ---------------- loss chunks ----------------
            partials = sb.tile([K, NCH], F32)
            for k in range(NCH):
                dP = ps.tile([K, CH], F32, tag="dpsum", bufs=3, name=f"dP{k}")
                nc.tensor.matmul(dP[:], rts[:], ft[:, k * CH:(k + 1) * CH],
                                 start=True, stop=False)
                nc.tensor.matmul(dP[:], p128s[:], ft2[:, k * CH:(k + 1) * CH],
                                 start=False, stop=True)
                mskd = sb.tile([K, CH], F32, tag="mskd", bufs=3, name=f"m{k}")
                nc.scalar.activation(mskd[:], dP[:], Act.Relu, bias=qm16[:])
                scr = sb.tile([K, CH], BF16, tag="scr", bufs=3, name=f"s{k}")
                nc.vector.scalar_tensor_tensor(scr[:], mskd[:], 0.0,
                                               eohts[:, k * CH:(k + 1) * CH],
                                               op0=Alu.add, op1=Alu.mult,
                                               accum_out=partials[:, k:k + 1])

            # ---------------- final reduction ----------------
            numP = ps.tile([1, NCH], F32)
            nc.tensor.matmul(numP[:], ones128[0:K, :], partials[:],
                             start=True, stop=True)
            num = sb.tile([1, 1], F32)
            nc.vector.tensor_reduce(num[:], numP[:], axis=AxX, op=Alu.add)
            dv = sb.tile([1, K], F32)
            nc.vector.tensor_tensor(dv[:], cntf, vmask[:], op=Alu.mult)
            den = sb.tile([1, 1], F32)
            nc.vector.tensor_reduce(den[:], dv[:], axis=AxX, op=Alu.add)
            nc.vector.tensor_scalar(den[:], den[:], 1.0, None, op0=Alu.max)
            nc.vector.reciprocal(den[:], den[:])
            resS = sb.tile([1, 1], F32)
            nc.vector.tensor_tensor(resS[:], num[:], den[:], op=Alu.mult)
            nc.sync.dma_start(res, resS[:])

    nc.compile()
    _CACHE["nc"] = nc
    return nc


def _make_in_maps(features, labels):
    feats = np.ascontiguousarray(np.asarray(features, dtype=np.float32))
    lab = np.ascontiguousarray(np.asarray(labels)).astype(np.int64)
    bf = ml_dtypes.bfloat16

    oh = (lab[:, None] == np.arange(K, dtype=np.int64)[None, :]).astype(bf)  # (N, K)
    one = {
        "fhr": np.ascontiguousarray(
            feats.reshape(T, 128, D).transpose(1, 0, 2).reshape(128, T * D)
        ).astype(bf),
        "ftr": np.ascontiguousarray(feats.T).astype(bf),
        "eohr": np.ascontiguousarray(
            oh.reshape(T, 128, K).transpose(1, 0, 2).reshape(128, T * K)),
        "eoht": np.ascontiguousarray(oh.T),
    }
    return [dict(one) for _ in range(NCORES)]


def kernel(features, labels):
    nc = _build()
    in_maps = _make_in_maps(features, labels)
    out = run_bass_kernel_spmd(nc, in_maps, core_ids=list(range(NCORES)))
    return np.float32(out.results[0]["res"][0, 0])
